# revision 21
# baseline (speedup 1.0000x reference)
"""Multi-head attention (L=2048, EMB=1024, H=16, D=64) on 8 TRN2 NeuronCores.

Tensor-parallel over heads: core i owns heads {2i, 2i+1} (a 128-row block of
Wq/Wk/Wv and a 128-column block of Wo). Each core computes its two heads'
attention plus its partial output projection; the host sums the 8 partials.

Device-side layout is fully transposed (scores^T = [m, l]) so no on-device
transposes are needed:
  QT[d, l] = (Wq_shard @ q^T)        lhsT = (Wq_shard/8)^T, rhs = q^T
  KT[d, l] = (Wk_shard @ k^T)
  V [m, d] = (v @ Wv_shard^T)        lhsT = v^T tile,       rhs = Wv_shard^T
  sT[m, l] = KT_h^T @ QT_h           (per head, contraction d=64)
  pT       = exp(sT) * keepT         (no max-subtraction: |s| <~ 9)
  attnT|Z  = [V_h | 1*64]^T @ pT     (ones cols 64:128 broadcast the softmax
                                      denominator Z onto PSUM rows 64:127)
  attnT/Z  = pa[0:64] * recip(pa[64:128])   local DVE, no DRAM bounce
  outT     = Wo_shard^T-block @ (attnT / Z)   bf16 partial, summed on host

All matmuls run in bf16 (fp32 PSUM accumulation); measured end-to-end
relative error vs the fp32 reference is ~0.6%.

Schedule notes (tuned against neuron-profile NTFF traces + the CoreSim
cost model):
- PE clock ramps 0.65 -> 1.2 -> 2.4 GHz with sustained use; every idle gap
  resets the ramp, so the whole schedule aims to keep the PE queue fed.
- The mask ships as fp8e4 (8 MB/core) and is upcast to bf16 in-flight by
  the gpsimd SWDGE DMA, so the DVE multiply keeps its 2x 16-bit rate.
- Output partials are stored bf16 (host sums in f64): halves store traffic.
- Critical-path input DMAs are split across rings: scalar gets {wq, q},
  sync gets {wk, k, wv, v, wo}, gpsimd gets mask fetches (half-tiles, one
  pass ahead), so the first scores matmul issues ~12us in.
- One-stage software pipeline on the PE queue: quad q's attn matmuls are
  emitted after quad q+1's scores, decoupling PE from the exp->mask-mult
  chain; exp is batched 3 key-tiles per ACTIVATE.
- Per-l-tile output projection is drip-fed one piece per quad into the
  next passes' streams as PE filler.
"""

import sys

for _p in ("/opt/trn_rl_repo",):
    if _p not in sys.path:
        sys.path.insert(0, _p)

from contextlib import ExitStack

import ml_dtypes
import numpy as np

import concourse.bass as bass
import concourse.tile as tile
from concourse import bacc, mybir
from concourse._compat import with_exitstack
from concourse.bass_utils import run_bass_kernel_spmd

BF16 = mybir.dt.bfloat16
FP8 = mybir.dt.float8e4
F32 = mybir.dt.float32
NPBF16 = ml_dtypes.bfloat16
NPFP8 = ml_dtypes.float8_e4m3

L = 2048
EMB = 1024
NHEAD = 16
HEAD_DIM = 64
NCORES = 8
HPC = NHEAD // NCORES  # heads per core = 2
ROWS = HPC * HEAD_DIM  # weight rows per core = 128
SCALE = HEAD_DIM ** -0.5

LT = 512               # l-tile (matmul free dim / PSUM bank)
NLT = L // LT          # 4
MT = 128               # m-tile (key-block on partitions)
NMT = L // MT          # 16
ET = 128               # contraction tile over EMB
NET = EMB // ET        # 8
JT = 128               # output-row tile
NJT = EMB // JT        # 8

QB = 2                 # psc tile m-capacity (PSUM banks per slot)
NSTEP = NMT // QB      # 8 mt-pair steps per (lt) with both heads interleaved
VROW = 128             # attn lhsT free dim: 64 V rows + 64 ones rows (Z bcast)


@with_exitstack
def _mha_kernel(ctx, tc, outT, qT, kT, vT, wqT, wkT, wvT, woT, maskT):
    nc = tc.nc

    const = ctx.enter_context(tc.tile_pool(name="const", bufs=1))
    ppool = ctx.enter_context(tc.tile_pool(name="ptiles", bufs=6))
    maskp = ctx.enter_context(tc.tile_pool(name="maskp", bufs=5))
    stage = ctx.enter_context(tc.tile_pool(name="stage", bufs=4))
    zpool = ctx.enter_context(tc.tile_pool(name="zpool", bufs=4))
    psc = ctx.enter_context(tc.tile_pool(name="psc", bufs=3, space="PSUM"))
    psa = ctx.enter_context(tc.tile_pool(name="psa", bufs=2, space="PSUM"))

    # ---- resident input tiles; DMAs split across rings in consumption order
    qTs = const.tile([128, NET, L], BF16, tag="qTs")
    kTs = const.tile([128, NET, L], BF16, tag="kTs")
    vTs = const.tile([128, NET, L], BF16, tag="vTs")
    wqs = const.tile([128, NET, ROWS], BF16, tag="wqs")
    wks = const.tile([128, NET, ROWS], BF16, tag="wks")
    wvs = const.tile([128, NET, ROWS], BF16, tag="wvs")
    wos = const.tile([128, EMB], BF16, tag="wos")  # [hd, j]
    q3 = qT.rearrange("(o p) l -> p o l", p=128)
    k3 = kT.rearrange("(o p) l -> p o l", p=128)
    v3 = vT.rearrange("(o p) l -> p o l", p=128)
    mask3 = maskT.rearrange("h (mo p) l -> h p mo l", p=128)
    out3 = outT.rearrange("(b p) l -> p b l", p=128)

    def chunk(eng, dst, src3, lc):
        eng.dma_start(dst[:, :, bass.ts(lc, LT)], src3[:, :, bass.ts(lc, LT)])

    # scalar ring: q-side critical path, then q tail; nothing else ever
    # (keeps the ACT engine free for exp). q0/k0 split in et-halves so the
    # prologue projections pipeline with their own loads.
    nc.scalar.dma_start(wqs[:], wqT[:])
    nc.scalar.dma_start(qTs[:, 0:4, 0:LT], q3[:, 0:4, 0:LT])
    nc.scalar.dma_start(qTs[:, 4:8, 0:LT], q3[:, 4:8, 0:LT])
    chunk(nc.scalar, qTs, q3, 1)
    # q chunks 2,3 are issued mid-lt0 (see step loop) so their transfers
    # don't compete with lt0's k/v/mask traffic
    # sync ring: k/v-side critical path, then bulk
    nc.sync.dma_start(wks[:], wkT[:])
    nc.sync.dma_start(kTs[:, 0:4, 0:LT], k3[:, 0:4, 0:LT])
    nc.sync.dma_start(kTs[:, 4:8, 0:LT], k3[:, 4:8, 0:LT])
    nc.sync.dma_start(wvs[:], wvT[:])
    chunk(nc.sync, vTs, v3, 0)
    for lc in range(1, NLT):
        chunk(nc.sync, kTs, k3, lc)
        chunk(nc.sync, vTs, v3, lc)
    nc.sync.dma_start(wos[:], woT[:])

    state = {}

    def mask_fetch(lt, h, half):
        mc = maskp.tile([128, 8, LT], BF16, tag="maskc",
                        name=f"maskc_{lt}_{h}_{half}")
        nc.gpsimd.dma_start(
            mc[:], mask3[h, :, 8 * half : 8 * half + 8, bass.ts(lt, LT)]
        )
        state[lt, h, half] = mc

    # hold the gpsimd mask queue behind the critical q0a load so the first
    # mask transfer doesn't steal DMA bandwidth from the q0/k0 critical path
    # (masks aren't consumed until ~15us in)
    gate = const.tile([1, 4], BF16, tag="gate")
    nc.gpsimd.tensor_copy(out=gate[:], in_=qTs[0:1, 0, 0:4])
    mask_fetch(0, 0, 0)
    mask_fetch(0, 1, 0)
    mask_fetch(0, 0, 1)
    mask_fetch(0, 1, 1)

    QTb = const.tile([128, L], BF16, tag="QTb")
    KTb = const.tile([128, L], BF16, tag="KTb")
    vaug = const.tile([128, HPC, NMT, VROW], BF16, tag="vaug")
    nc.vector.memset(vaug[:, :, :, HEAD_DIM:VROW], 1.0)
    attnTb = const.tile([128, L], BF16, tag="attnTb")

    def qk_proj(dst, w, x, lt, ps, use_act):
        for et in range(NET):
            nc.tensor.matmul(
                ps[:],
                lhsT=w[:, et, :],
                rhs=x[:, et, bass.ts(lt, LT)],
                start=(et == 0),
                stop=(et == NET - 1),
            )
        if use_act:
            nc.scalar.copy(out=dst[:, bass.ts(lt, LT)], in_=ps[:])
        else:
            nc.vector.tensor_copy(out=dst[:, bass.ts(lt, LT)], in_=ps[:])

    # NOTE: PSUM accumulation groups are bank-granular; only one open group
    # per bank at a time (interleaving two in a bank corrupts both).
    def v_proj(mt):
        ps = psc.tile([128, QB, LT], F32, tag="psc", name="ps_v")[:, 0, :]
        for et in range(NET):
            nc.tensor.matmul(
                ps[:, :ROWS],
                lhsT=vTs[:, et, bass.ts(mt, MT)],
                rhs=wvs[:, et, :],
                start=(et == 0),
                stop=(et == NET - 1),
            )
        for h in range(HPC):
            nc.vector.tensor_copy(
                out=vaug[:, h, mt, 0:HEAD_DIM],
                in_=ps[:, bass.ts(h, HEAD_DIM)],
            )

    def k_proj(lc):
        ps = psc.tile([128, QB, LT], F32, tag="psc", name="ps_k")[:, 0, :]
        qk_proj(KTb, wks, kTs, lc, ps, use_act=True)

    def q_proj(lc):
        ps = psc.tile([128, QB, LT], F32, tag="psc", name="ps_q")[:, 0, :]
        qk_proj(QTb, wqs, qTs, lc, ps, use_act=False)

    # ---- prologue: Q(lt0) and K(chunk0) projections on separate psc slots,
    # emitted in et-half blocks interleaved in DMA-arrival order
    ps_q0 = psc.tile([128, QB, LT], F32, tag="psc", name="ps_q0")[:, 0, :]
    ps_k0 = psc.tile([128, QB, LT], F32, tag="psc", name="ps_k0")[:, 0, :]
    for lo, hi in ((0, 4), (4, 8)):
        for ps, w, x in ((ps_q0, wqs, qTs), (ps_k0, wks, kTs)):
            for et in range(lo, hi):
                nc.tensor.matmul(
                    ps[:],
                    lhsT=w[:, et, :],
                    rhs=x[:, et, 0:LT],
                    start=(et == 0),
                    stop=(et == NET - 1),
                )
    nc.scalar.copy(out=QTb[:, 0:LT], in_=ps_q0[:])
    nc.scalar.copy(out=KTb[:, 0:LT], in_=ps_k0[:])

    # PE filler by (lt, step): K chunk b feeds scores of step 2b; v(mt)
    # feeds the attn matmuls of step mt//2 which are emitted at step
    # mt//2 + 1. Q projections for lt2/lt3 are deferred out of lt0 (and
    # their loads issued mid-lt0) to relieve lt0's DMA-bandwidth floor.
    filler = {
        (0, 0): [lambda: v_proj(0), lambda: v_proj(1)],
        (0, 1): [lambda: v_proj(2), lambda: v_proj(3)],
        (0, 2): [lambda: k_proj(1), lambda: v_proj(4), lambda: v_proj(5)],
        (0, 3): [lambda: v_proj(6), lambda: v_proj(7), lambda: q_proj(1)],
        (0, 4): [lambda: k_proj(2), lambda: v_proj(8), lambda: v_proj(9),
                 lambda: chunk(nc.scalar, qTs, q3, 2)],
        (0, 5): [lambda: v_proj(10), lambda: v_proj(11)],
        (0, 6): [lambda: k_proj(3), lambda: v_proj(12), lambda: v_proj(13),
                 lambda: chunk(nc.scalar, qTs, q3, 3)],
        (0, 7): [lambda: v_proj(14), lambda: v_proj(15)],
        (1, 2): [lambda: q_proj(2)],
        (2, 2): [lambda: q_proj(3)],
    }

    # deferred per-l-tile output projection (2 jt per psc slot, one per
    # bank — sequential groups in separate banks), drip-fed as PE filler
    pending = []

    def piece_outproj(lt, jp):
        def go():
            ls = bass.ts(lt, LT)
            ps = psc.tile([128, QB, LT], F32, tag="psc", name="ps_out")
            st = stage.tile([128, 2, LT], BF16, tag="st", name=f"st_{lt}_{jp}")
            for i in range(2):
                nc.tensor.matmul(
                    ps[:, i, :],
                    lhsT=wos[:, bass.ts(2 * jp + i, JT)],
                    rhs=attnTb[:, ls],
                    start=True,
                    stop=True,
                )
            nc.vector.tensor_copy(out=st[:, 0, :], in_=ps[:, 0, :])
            nc.scalar.copy(out=st[:, 1, :], in_=ps[:, 1, :])
            nc.sync.dma_start(out3[:, 2 * jp : 2 * jp + 2, ls], st[:])
        return go

    for lt in range(NLT):
        ls = bass.ts(lt, LT)
        pa = [psa.tile([128, LT], F32, tag="psa", name=f"psa_{lt}_{h}")
              for h in range(HPC)]
        prev_attn = [None, None]
        for s in range(NSTEP):
            mt0 = QB * s
            half = 0 if mt0 < 8 else 1
            j0 = mt0 - 8 * half
            for f in filler.get((lt, s), ()):
                f()
            # prefetch next lt's mask halves across steps 3..6
            if lt + 1 < NLT and 3 <= s <= 6:
                mask_fetch(lt + 1, (s - 3) % 2, (s - 3) // 2)
            for h in range(HPC):
                hd = bass.ts(h, HEAD_DIM)
                maskc = state[lt, h, half]
                ss = psc.tile([128, QB, LT], F32, tag="psc", name="ss")
                for i in range(QB):
                    nc.tensor.matmul(
                        ss[:, i, :],
                        lhsT=KTb[hd, bass.ts(mt0 + i, MT)],
                        rhs=QTb[hd, ls],
                        start=True,
                        stop=True,
                    )
                # software pipeline: this head's previous attn matmuls are
                # emitted after this step's scores
                if prev_attn[h] is not None:
                    prev_attn[h]()
                pT = ppool.tile([128, QB, LT], BF16, tag="pT", name="pT")
                nc.scalar.activation(
                    pT[:], ss[:], mybir.ActivationFunctionType.Exp
                )
                nc.vector.tensor_mul(
                    out=pT[:], in0=pT[:], in1=maskc[:, j0 : j0 + QB, :],
                )

                def make_attn(h=h, mt0=mt0, pT=pT):
                    def go():
                        for i in range(QB):
                            mt = mt0 + i
                            nc.tensor.matmul(
                                pa[h][:],
                                lhsT=vaug[:, h, mt, :],
                                rhs=pT[:, i, :],
                                start=(mt == 0),
                                stop=(mt == NMT - 1),
                            )
                    return go

                prev_attn[h] = make_attn()
            # drip one deferred out-proj piece at the END of steps 1,3,5,6:
            # late enough that the previous lt's normalize chain is done,
            # and never at step 7 — the piece's psc slot would make the next
            # lt's first scores wait on the piece's ACT copy (which queues
            # behind this step's exp)
            if pending and s in (1, 3, 5, 6):
                pending.pop(0)()
        for h in range(HPC):
            prev_attn[h]()
            hd = bass.ts(h, HEAD_DIM)
            # local softmax normalize: rows 64:127 of pa all hold Z
            # (emitted before the other head's last attn so DVE overlaps PE)
            # custom-DVE bitwise ops misread PSUM: stage Z to SBUF (ACT copy)
            # before the approx reciprocal
            zsb = zpool.tile([64, LT], F32, tag="zsb", name=f"zsb_{lt}_{h}")
            nc.scalar.copy(out=zsb[:], in_=pa[h][64:128, :])
            zinv = zpool.tile([64, LT], F32, tag="zinv", name=f"zinv_{lt}_{h}")
            nc.vector.reciprocal_approx_fast(out=zinv[:], in_=zsb[:])
            nc.vector.tensor_mul(
                out=attnTb[hd, ls], in0=pa[h][0:HEAD_DIM, :], in1=zinv[:]
            )
        for jp in range(NJT // 2):
            pending.append(piece_outproj(lt, jp))

    while pending:
        pending.pop(0)()


_CACHE = {}


def _build():
    if "nc" in _CACHE:
        return _CACHE["nc"]
    nc = bacc.Bacc("TRN2", target_bir_lowering=False, debug=False,
                   num_devices=NCORES)
    qT = nc.dram_tensor("qT", [EMB, L], BF16, kind="ExternalInput").ap()
    kT = nc.dram_tensor("kT", [EMB, L], BF16, kind="ExternalInput").ap()
    vT = nc.dram_tensor("vT", [EMB, L], BF16, kind="ExternalInput").ap()
    wqT = nc.dram_tensor("wqT", [128, NET, ROWS], BF16, kind="ExternalInput").ap()
    wkT = nc.dram_tensor("wkT", [128, NET, ROWS], BF16, kind="ExternalInput").ap()
    wvT = nc.dram_tensor("wvT", [128, NET, ROWS], BF16, kind="ExternalInput").ap()
    woT = nc.dram_tensor("woT", [ROWS, EMB], BF16, kind="ExternalInput").ap()
    maskT = nc.dram_tensor("maskT", [HPC, L, L], FP8, kind="ExternalInput").ap()
    outT = nc.dram_tensor("outT", [EMB, L], BF16, kind="ExternalOutput").ap()

    with tile.TileContext(nc) as tc:
        _mha_kernel(tc, outT, qT, kT, vT, wqT, wkT, wvT, woT, maskT)
    nc.compile()
    _CACHE["nc"] = nc
    return nc


def _pack_w(w):
    # [ROWS, EMB] -> w.T [EMB, ROWS] -> [128, NET, ROWS] with e = o*128+p
    return np.ascontiguousarray(
        w.T.reshape(NET, 128, ROWS).transpose(1, 0, 2)
    ).astype(NPBF16)


def _prep_in_maps(q, k, v, mask, Wq, Wk, Wv, Wo):
    qT = np.ascontiguousarray(q.T).astype(NPBF16)
    kT = np.ascontiguousarray(k.T).astype(NPBF16)
    vT = np.ascontiguousarray(v.T).astype(NPBF16)
    in_maps = []
    for c in range(NCORES):
        rows = slice(c * ROWS, (c + 1) * ROWS)
        in_maps.append({
            "qT": qT,
            "kT": kT,
            "vT": vT,
            "wqT": _pack_w(Wq[rows] * SCALE),
            "wkT": _pack_w(Wk[rows]),
            "wvT": _pack_w(Wv[rows]),
            "woT": np.ascontiguousarray(Wo[:, rows].T).astype(NPBF16),
            "maskT": np.ascontiguousarray(
                (~mask[c * HPC : (c + 1) * HPC]).swapaxes(1, 2)
            ).astype(NPFP8),
        })
    return in_maps


def run(q, k, v, mask, Wq, Wk, Wv, Wo, **spmd_kwargs):
    nc = _build()
    in_maps = _prep_in_maps(q, k, v, mask, Wq, Wk, Wv, Wo)
    res = run_bass_kernel_spmd(nc, in_maps, list(range(NCORES)), **spmd_kwargs)
    outT = np.zeros((EMB, L), np.float64)
    for r in res.results:
        outT += np.asarray(r["outT"]).astype(np.float64)
    out = np.ascontiguousarray(outT.T).astype(np.float32)
    return out, res


def kernel(q, k, v, mask, Wq, Wk, Wv, Wo):
    q, k, v = (np.asarray(x, np.float32) for x in (q, k, v))
    Wq, Wk, Wv, Wo = (np.asarray(x, np.float32) for x in (Wq, Wk, Wv, Wo))
    mask = np.asarray(mask, bool)
    out, _ = run(q, k, v, mask, Wq, Wk, Wv, Wo)
    return out
